# revision 3
# baseline (speedup 1.0000x reference)
"""Multi-LoRA batched einsum kernel for Trainium2 (8 NeuronCores).

Computes: out[b,s,r] = sum_h x[b,s,h] * weight[adapter_ids[b], r, h]
  x:       [8, 2048, 8192] f32
  weight:  [1024, 16, 8192] f32   (adapter pool)
  adapter_ids: [8] i32
  out:     [8, 2048, 16] f32

Distribution (tensor-parallel over the hidden dim, per the sharding hint):
  - core d receives the H-slice [d*1024, (d+1)*1024) of x (laid out [B, h, S]
    so the contraction dim is on partitions) and the same H-slice of the
    full adapter pool.
  - on-device, each core gathers the 8 active adapters out of its pool slice
    with an indirect DMA, PE-transposes them into [h, r] layout, then runs
    per-batch matmuls accumulating the 1024-deep local contraction in PSUM.
  - the host sums the 8 partial outputs (allreduce equivalent) and restores
    the [B, S, R] layout.
"""

import numpy as np

B, S, H, R, POOL = 8, 2048, 8192, 16, 1024
NCORES = 8
HS = H // NCORES  # 1024: per-core hidden slice
K = HS // 128     # 8 contraction chunks of 128
NS = 4            # output column chunks
SW = S // NS      # 512 (max fp32 matmul moving dim)

MM_DT = "float32"  # matmul dtype: "float32" (exact) or "float32r" (fast)

_cache: dict = {}


def _build(mm_dt_name: str):
    import concourse.bass as bass
    import concourse.mybir as mybir
    import concourse.tile as tile
    from concourse import bacc
    from concourse.masks import make_identity

    f32 = mybir.dt.float32
    i32 = mybir.dt.int32
    mm_dt = getattr(mybir.dt, mm_dt_name)

    nc = bacc.Bacc("TRN2", target_bir_lowering=False)
    xT = nc.dram_tensor("xT", [B, HS, S], mm_dt, kind="ExternalInput")
    pool = nc.dram_tensor("pool", [POOL, R, HS], mm_dt, kind="ExternalInput")
    widx = nc.dram_tensor("widx", [B, R, 1], i32, kind="ExternalInput")
    out = nc.dram_tensor("out", [B, R, S], f32, kind="ExternalOutput")

    with tile.TileContext(nc) as tc:
        with (
            tc.tile_pool(name="const", bufs=1) as cpool,
            tc.tile_pool(name="wload", bufs=2) as wload,
            tc.tile_pool(name="wps", bufs=2, space="PSUM") as wps,
            tc.tile_pool(name="xs", bufs=2) as xs,
            tc.tile_pool(name="mps", bufs=6, space="PSUM") as mps,
            tc.tile_pool(name="osb", bufs=2) as osb,
        ):
            ident = cpool.tile([R, R], mm_dt, name="ident")
            make_identity(nc, ident[:])

            # Gather the 8 active adapters and transpose to [h, r] layout.
            # wT[:, b*K + k, :] is the [128, 16] stationary operand for
            # batch b, contraction chunk k.
            wT = cpool.tile([128, B * K, R], mm_dt, name="wT")
            pool_rows = pool[:].rearrange("a r h -> (a r) h")
            for b in range(B):
                idx_t = wload.tile([R, 1], i32, tag="idx", name=f"idx_{b}")
                nc.sync.dma_start(idx_t[:], widx[b])
                w_b = wload.tile([R, HS], mm_dt, tag="wb", name=f"wb_{b}")
                nc.gpsimd.indirect_dma_start(
                    out=w_b[:],
                    out_offset=None,
                    in_=pool_rows,
                    in_offset=bass.IndirectOffsetOnAxis(ap=idx_t[:, :1], axis=0),
                )
                for k in range(K):
                    ps = wps.tile([128, R], f32, tag="wt", name=f"wt_{b}_{k}")
                    nc.tensor.transpose(
                        ps[:], w_b[:, k * 128:(k + 1) * 128], ident[:]
                    )
                    nc.vector.tensor_copy(wT[:, b * K + k, :], ps[:])

            # Stream x through: one 8 MiB DMA per batch, 32 matmuls each.
            xTr = xT[:].rearrange("b (k p) s -> b p k s", p=128)
            for b in range(B):
                x_t = xs.tile([128, K, S], mm_dt, tag="xt", name=f"xt_{b}")
                nc.sync.dma_start(x_t[:], xTr[b])
                psums = [
                    mps.tile([R, SW], f32, tag="mm", name=f"mm_{b}_{n}")
                    for n in range(NS)
                ]
                for k in range(K):
                    for n in range(NS):
                        nc.tensor.matmul(
                            psums[n][:],
                            lhsT=wT[:, b * K + k, :],
                            rhs=x_t[:, k, n * SW:(n + 1) * SW],
                            start=(k == 0),
                            stop=(k == K - 1),
                        )
                o_t = osb.tile([R, S], f32, tag="ot", name=f"ot_{b}")
                for n in range(NS):
                    nc.scalar.copy(o_t[:, n * SW:(n + 1) * SW], psums[n][:])
                nc.sync.dma_start(out[b], o_t[:])
    nc.compile()
    return nc


def _get_nc():
    if MM_DT not in _cache:
        _cache[MM_DT] = _build(MM_DT)
    return _cache[MM_DT]


def _shard_inputs(x, weight, adapter_ids):
    """Host-side sharding: H-slice per core, contraction dim onto partitions."""
    x = np.ascontiguousarray(np.asarray(x, dtype=np.float32))
    weight = np.ascontiguousarray(np.asarray(weight, dtype=np.float32))
    ids = np.asarray(adapter_ids).astype(np.int64)

    # [NCORES, B, HS, S]: per-core slice of x, transposed so h is leading
    xT = np.ascontiguousarray(
        x.reshape(B, S, NCORES, HS).transpose(2, 0, 3, 1)
    )
    # [NCORES, POOL, R, HS]: per-core H-slice of the adapter pool
    pool_sh = np.ascontiguousarray(
        weight.reshape(POOL, R, NCORES, HS).transpose(2, 0, 1, 3)
    )
    # row indices into the [(POOL R), HS] flat view: id*R + r
    idx = (ids[:, None] * R + np.arange(R)[None, :]).astype(np.int32)
    idx = np.ascontiguousarray(idx.reshape(B, R, 1))

    return [
        {"xT": xT[d], "pool": pool_sh[d], "widx": idx}
        for d in range(NCORES)
    ]


def _run(x, weight, adapter_ids, trace=False, trace_cores=None):
    from concourse.bass_utils import run_bass_kernel_spmd

    nc = _get_nc()
    in_maps = _shard_inputs(x, weight, adapter_ids)
    res = run_bass_kernel_spmd(
        nc,
        in_maps,
        core_ids=list(range(NCORES)),
        trace=trace,
        trace_cores=trace_cores,
    )
    # Host unshard: sum the 8 partial contractions, restore [B, S, R]
    acc = np.zeros((B, R, S), dtype=np.float64)
    for r in res.results:
        acc += r["out"]
    out = np.ascontiguousarray(acc.transpose(0, 2, 1).astype(np.float32))
    return out, res


def kernel(x, weight, weight_active, adapter_ids):
    # weight_active is all-zeros scratch fully overwritten by the reference's
    # dynamic_update_slice; it does not affect the output.
    out, _ = _run(x, weight, adapter_ids, trace=False)
    return out


# revision 6
# speedup vs baseline: 1.2376x; 1.2376x over previous
"""Multi-LoRA batched einsum kernel for Trainium2 (8 NeuronCores).

Computes: out[b,s,r] = sum_h x[b,s,h] * weight[adapter_ids[b], r, h]
  x:       [8, 2048, 8192] f32
  weight:  [1024, 16, 8192] f32   (adapter pool)
  adapter_ids: [8] i32
  out:     [8, 2048, 16] f32

Distribution (tensor-parallel over the hidden dim, per the sharding hint):
  - core d receives the H-slice [d*1024, (d+1)*1024) of x (laid out [B, h, S]
    so the contraction dim is on partitions) and the same H-slice of the
    full adapter pool.
  - on-device, each core gathers the 8 active adapters out of its pool slice
    with an indirect DMA, PE-transposes them into [h, r] layout, then runs
    per-batch matmuls accumulating the 1024-deep local contraction in PSUM.
  - the host sums the 8 partial outputs (allreduce equivalent) and restores
    the [B, S, R] layout.
"""

import numpy as np

B, S, H, R, POOL = 8, 2048, 8192, 16, 1024
NCORES = 8
HS = H // NCORES  # 1024: per-core hidden slice
K = HS // 128     # 8 contraction chunks of 128
NS = 4            # output column chunks
SW = S // NS      # 512 (max fp32 matmul moving dim)

MM_DT = "float32r"  # matmul dtype: "float32" (exact) or "float32r" (fast)

_cache: dict = {}


def _build(mm_dt_name: str):
    import concourse.bass as bass
    import concourse.mybir as mybir
    import concourse.tile as tile
    from concourse import bacc
    from concourse.masks import make_identity

    f32 = mybir.dt.float32
    i32 = mybir.dt.int32
    mm_dt = getattr(mybir.dt, mm_dt_name)

    nc = bacc.Bacc("TRN2", target_bir_lowering=False)
    xT = nc.dram_tensor("xT", [B, HS, S], mm_dt, kind="ExternalInput")
    pool = nc.dram_tensor("pool", [POOL, R, HS], f32, kind="ExternalInput")
    widx = nc.dram_tensor("widx", [B, R, 1], i32, kind="ExternalInput")
    out = nc.dram_tensor("out", [B, R, S], f32, kind="ExternalOutput")

    with tile.TileContext(nc) as tc:
        with (
            tc.tile_pool(name="const", bufs=1) as cpool,
            tc.tile_pool(name="wload", bufs=2) as wload,
            tc.tile_pool(name="wps", bufs=2, space="PSUM") as wps,
            tc.tile_pool(name="xs", bufs=2) as xs,
            tc.tile_pool(name="mps", bufs=6, space="PSUM") as mps,
            tc.tile_pool(name="osb", bufs=2) as osb,
        ):
            ident = cpool.tile([R, R], f32, name="ident")
            make_identity(nc, ident[:])

            # Gather the 8 active adapters and transpose to [h, r] layout.
            # wT[:, b*K + k, :] is the [128, 16] stationary operand for
            # batch b, contraction chunk k.
            wT = cpool.tile([128, B * K, R], mm_dt, name="wT")
            pool_rows = pool[:].rearrange("a r h -> (a r) h")
            for b in range(B):
                idx_t = wload.tile([R, 1], i32, tag="idx", name=f"idx_{b}")
                nc.sync.dma_start(idx_t[:], widx[b])
                w_b = wload.tile([R, HS], f32, tag="wb", name=f"wb_{b}")
                nc.gpsimd.indirect_dma_start(
                    out=w_b[:],
                    out_offset=None,
                    in_=pool_rows,
                    in_offset=bass.IndirectOffsetOnAxis(ap=idx_t[:, :1], axis=0),
                )
                for k in range(K):
                    ps = wps.tile([128, R], f32, tag="wt", name=f"wt_{b}_{k}")
                    nc.tensor.transpose(
                        ps[:], w_b[:, k * 128:(k + 1) * 128], ident[:]
                    )
                    nc.vector.tensor_copy(wT[:, b * K + k, :], ps[:])

            # Stream x through: one 8 MiB DMA per batch, 32 matmuls each.
            xTr = xT[:].rearrange("b (k p) s -> b p k s", p=128)
            for b in range(B):
                x_t = xs.tile([128, K, S], mm_dt, tag="xt", name=f"xt_{b}")
                nc.sync.dma_start(x_t[:], xTr[b])
                psums = [
                    mps.tile([R, SW], f32, tag="mm", name=f"mm_{b}_{n}")
                    for n in range(NS)
                ]
                for k in range(K):
                    for n in range(NS):
                        nc.tensor.matmul(
                            psums[n][:],
                            lhsT=wT[:, b * K + k, :],
                            rhs=x_t[:, k, n * SW:(n + 1) * SW],
                            start=(k == 0),
                            stop=(k == K - 1),
                        )
                o_t = osb.tile([R, S], f32, tag="ot", name=f"ot_{b}")
                for n in range(NS):
                    nc.scalar.copy(o_t[:, n * SW:(n + 1) * SW], psums[n][:])
                nc.sync.dma_start(out[b], o_t[:])
    nc.compile()
    return nc


def _get_nc():
    if MM_DT not in _cache:
        _cache[MM_DT] = _build(MM_DT)
    return _cache[MM_DT]


def _shard_inputs(x, weight, adapter_ids):
    """Host-side sharding: H-slice per core, contraction dim onto partitions."""
    x = np.ascontiguousarray(np.asarray(x, dtype=np.float32))
    weight = np.ascontiguousarray(np.asarray(weight, dtype=np.float32))
    ids = np.asarray(adapter_ids).astype(np.int64)

    # [NCORES, B, HS, S]: per-core slice of x, transposed so h is leading
    xT = np.ascontiguousarray(
        x.reshape(B, S, NCORES, HS).transpose(2, 0, 3, 1)
    )
    # [NCORES, POOL, R, HS]: per-core H-slice of the adapter pool
    pool_sh = np.ascontiguousarray(
        weight.reshape(POOL, R, NCORES, HS).transpose(2, 0, 1, 3)
    )
    # row indices into the [(POOL R), HS] flat view: id*R + r
    idx = (ids[:, None] * R + np.arange(R)[None, :]).astype(np.int32)
    idx = np.ascontiguousarray(idx.reshape(B, R, 1))

    return [
        {"xT": xT[d], "pool": pool_sh[d], "widx": idx}
        for d in range(NCORES)
    ]


def _run(x, weight, adapter_ids, trace=False, trace_cores=None):
    from concourse.bass_utils import run_bass_kernel_spmd

    nc = _get_nc()
    in_maps = _shard_inputs(x, weight, adapter_ids)
    res = run_bass_kernel_spmd(
        nc,
        in_maps,
        core_ids=list(range(NCORES)),
        trace=trace,
        trace_cores=trace_cores,
    )
    # Host unshard: sum the 8 partial contractions, restore [B, S, R]
    acc = np.zeros((B, R, S), dtype=np.float64)
    for r in res.results:
        acc += r["out"]
    out = np.ascontiguousarray(acc.transpose(0, 2, 1).astype(np.float32))
    return out, res


def kernel(x, weight, weight_active, adapter_ids):
    # weight_active is all-zeros scratch fully overwritten by the reference's
    # dynamic_update_slice; it does not affect the output.
    out, _ = _run(x, weight, adapter_ids, trace=False)
    return out


# revision 11
# speedup vs baseline: 1.3759x; 1.1118x over previous
"""Multi-LoRA batched einsum kernel for Trainium2 (8 NeuronCores).

Computes: out[b,s,r] = sum_h x[b,s,h] * weight[adapter_ids[b], r, h]
  x:       [8, 2048, 8192] f32
  weight:  [1024, 16, 8192] f32   (adapter pool)
  adapter_ids: [8] i32
  out:     [8, 2048, 16] f32

Distribution (tensor-parallel over the hidden dim, per the sharding hint):
  - core d receives the H-slice [d*1024, (d+1)*1024) of x (laid out [B, h, S]
    so the contraction dim is on partitions) and the same H-slice of the
    full adapter pool.
  - on-device, each core gathers the 8 active adapters out of its pool slice
    with an indirect DMA, PE-transposes them into [h, r] layout, then runs
    per-batch matmuls accumulating the 1024-deep local contraction in PSUM.
  - the host sums the 8 partial outputs (allreduce equivalent) and restores
    the [B, S, R] layout.
"""

import numpy as np

B, S, H, R, POOL = 8, 2048, 8192, 16, 1024
NCORES = 8
HS = H // NCORES  # 1024: per-core hidden slice
K = HS // 128     # 8 contraction chunks of 128
NS = 4            # output column chunks
SW = S // NS      # 512 (max fp32 matmul moving dim)
XC = 4            # x-load chunks per batch (K/XC k-chunks per load)
KC = K // XC      # k-chunks per x-load

MM_DT = "float32r"  # matmul dtype: "float32" (exact) or "float32r" (fast)

_cache: dict = {}


def _build(mm_dt_name: str):
    import concourse.bass as bass
    import concourse.mybir as mybir
    import concourse.tile as tile
    from concourse import bacc
    from concourse.masks import make_identity

    f32 = mybir.dt.float32
    i32 = mybir.dt.int32
    mm_dt = getattr(mybir.dt, mm_dt_name)

    nc = bacc.Bacc("TRN2", target_bir_lowering=False)
    # xT layout [B, p, K, S]: partition-major so each partition's chunk is
    # one contiguous DRAM run (h = k*128 + p)
    xT = nc.dram_tensor("xT", [B, 128, K, S], mm_dt, kind="ExternalInput")
    pool = nc.dram_tensor("pool", [POOL, R, HS], f32, kind="ExternalInput")
    widx = nc.dram_tensor("widx", [B, R, 1], i32, kind="ExternalInput")
    out = nc.dram_tensor("out", [B, R, S], f32, kind="ExternalOutput")

    with tile.TileContext(nc) as tc:
        with (
            tc.tile_pool(name="const", bufs=1) as cpool,
            tc.tile_pool(name="wload", bufs=2) as wload,
            tc.tile_pool(name="wps", bufs=2, space="PSUM") as wps,
            tc.tile_pool(name="xs", bufs=6) as xs,
            tc.tile_pool(name="mps", bufs=6, space="PSUM") as mps,
            tc.tile_pool(name="osb", bufs=2) as osb,
        ):
            ident = cpool.tile([R, R], f32, name="ident")
            make_identity(nc, ident[:])

            # Gather the 8 active adapters and transpose to [h, r] layout.
            # wT[:, b*K + k, :] is the [128, 16] stationary operand for
            # batch b, contraction chunk k.
            wT = cpool.tile([128, B * K, R], mm_dt, name="wT")
            pool_rows = pool[:].rearrange("a r h -> (a r) h")
            for b in range(B):
                idx_t = wload.tile([R, 1], i32, tag="idx", name=f"idx_{b}")
                nc.sync.dma_start(idx_t[:], widx[b])
                w_b = wload.tile([R, HS], f32, tag="wb", name=f"wb_{b}")
                nc.gpsimd.indirect_dma_start(
                    out=w_b[:],
                    out_offset=None,
                    in_=pool_rows,
                    in_offset=bass.IndirectOffsetOnAxis(ap=idx_t[:, :1], axis=0),
                )
                for k in range(K):
                    ps = wps.tile([128, R], f32, tag="wt", name=f"wt_{b}_{k}")
                    nc.tensor.transpose(
                        ps[:], w_b[:, k * 128:(k + 1) * 128], ident[:]
                    )
                    nc.vector.tensor_copy(wT[:, b * K + k, :], ps[:])

            # Stream x through in 2 MiB chunks (XC per batch), matmuls
            # accumulate the local contraction into 4 PSUM column strips.
            for b in range(B):
                psums = [
                    mps.tile([R, SW], f32, tag="mm", name=f"mm_{b}_{n}")
                    for n in range(NS)
                ]
                for c in range(XC):
                    x_t = xs.tile([128, KC, S], mm_dt, tag="xt",
                                  name=f"xt_{b}_{c}")
                    nc.sync.dma_start(
                        x_t[:], xT[b][:, c * KC:(c + 1) * KC, :]
                    )
                    for kc in range(KC):
                        k = c * KC + kc
                        for n in range(NS):
                            nc.tensor.matmul(
                                psums[n][:],
                                lhsT=wT[:, b * K + k, :],
                                rhs=x_t[:, kc, n * SW:(n + 1) * SW],
                                start=(k == 0),
                                stop=(k == K - 1),
                            )
                o_t = osb.tile([R, S], f32, tag="ot", name=f"ot_{b}")
                for n in range(NS):
                    nc.scalar.copy(o_t[:, n * SW:(n + 1) * SW], psums[n][:])
                nc.sync.dma_start(out[b], o_t[:])
    nc.compile()
    return nc


def _get_nc():
    if MM_DT not in _cache:
        _cache[MM_DT] = _build(MM_DT)
    return _cache[MM_DT]


def _shard_inputs(x, weight, adapter_ids):
    """Host-side sharding: H-slice per core, contraction dim onto partitions."""
    x = np.ascontiguousarray(np.asarray(x, dtype=np.float32))
    weight = np.ascontiguousarray(np.asarray(weight, dtype=np.float32))
    ids = np.asarray(adapter_ids).astype(np.int64)

    # [NCORES, B, 128, K, S]: per-core H-slice of x, laid out so the
    # contraction dim is on partitions (h = k*128 + p) and each partition's
    # data is one contiguous DRAM run per chunk
    xT = np.ascontiguousarray(
        x.reshape(B, S, NCORES, K, 128).transpose(2, 0, 4, 3, 1)
    )
    # [NCORES, POOL, R, HS]: per-core H-slice of the adapter pool
    pool_sh = np.ascontiguousarray(
        weight.reshape(POOL, R, NCORES, HS).transpose(2, 0, 1, 3)
    )
    # row indices into the [(POOL R), HS] flat view: id*R + r
    idx = (ids[:, None] * R + np.arange(R)[None, :]).astype(np.int32)
    idx = np.ascontiguousarray(idx.reshape(B, R, 1))

    return [
        {"xT": xT[d], "pool": pool_sh[d], "widx": idx}
        for d in range(NCORES)
    ]


def _run(x, weight, adapter_ids, trace=False, trace_cores=None):
    from concourse.bass_utils import run_bass_kernel_spmd

    nc = _get_nc()
    in_maps = _shard_inputs(x, weight, adapter_ids)
    res = run_bass_kernel_spmd(
        nc,
        in_maps,
        core_ids=list(range(NCORES)),
        trace=trace,
        trace_cores=trace_cores,
    )
    # Host unshard: sum the 8 partial contractions, restore [B, S, R]
    acc = np.zeros((B, R, S), dtype=np.float64)
    for r in res.results:
        acc += r["out"]
    out = np.ascontiguousarray(acc.transpose(0, 2, 1).astype(np.float32))
    return out, res


def kernel(x, weight, weight_active, adapter_ids):
    # weight_active is all-zeros scratch fully overwritten by the reference's
    # dynamic_update_slice; it does not affect the output.
    out, _ = _run(x, weight, adapter_ids, trace=False)
    return out


# revision 13
# speedup vs baseline: 1.4962x; 1.0874x over previous
"""Multi-LoRA batched einsum kernel for Trainium2 (8 NeuronCores).

Computes: out[b,s,r] = sum_h x[b,s,h] * weight[adapter_ids[b], r, h]
  x:       [8, 2048, 8192] f32
  weight:  [1024, 16, 8192] f32   (adapter pool)
  adapter_ids: [8] i32
  out:     [8, 2048, 16] f32

Distribution (tensor-parallel over the hidden dim, per the sharding hint):
  - core d receives the H-slice [d*1024, (d+1)*1024) of x (laid out [B, h, S]
    so the contraction dim is on partitions) and the same H-slice of the
    full adapter pool.
  - on-device, each core gathers the 8 active adapters out of its pool slice
    with an indirect DMA, PE-transposes them into [h, r] layout, then runs
    per-batch matmuls accumulating the 1024-deep local contraction in PSUM.
  - the host sums the 8 partial outputs (allreduce equivalent) and restores
    the [B, S, R] layout.
"""

import numpy as np

B, S, H, R, POOL = 8, 2048, 8192, 16, 1024
NCORES = 8
HS = H // NCORES  # 1024: per-core hidden slice
K = HS // 128     # 8 contraction chunks of 128
NS = 4            # output column chunks
SW = S // NS      # 512 (max fp32 matmul moving dim)
XC = 4            # x-load chunks per batch (K/XC k-chunks per load)
KC = K // XC      # k-chunks per x-load

MM_DT = "float32r"  # matmul dtype: "float32" (exact) or "float32r" (fast)

_cache: dict = {}


def _build(mm_dt_name: str):
    import concourse.bass as bass
    import concourse.mybir as mybir
    import concourse.tile as tile
    from concourse import bacc
    from concourse.masks import make_identity

    f32 = mybir.dt.float32
    i32 = mybir.dt.int32
    mm_dt = getattr(mybir.dt, mm_dt_name)

    nc = bacc.Bacc("TRN2", target_bir_lowering=False)
    # xT layout [B, p, K, S]: partition-major so each partition's chunk is
    # one contiguous DRAM run (h = k*128 + p)
    xT = nc.dram_tensor("xT", [B, 128, K, S], mm_dt, kind="ExternalInput")
    pool = nc.dram_tensor("pool", [POOL, R, HS], f32, kind="ExternalInput")
    widx = nc.dram_tensor("widx", [B, R, 1], i32, kind="ExternalInput")
    out = nc.dram_tensor("out", [B, R, S], f32, kind="ExternalOutput")

    NCH = B * XC   # total x chunks
    WARM = 8       # chunk loads kept in flight ahead of compute

    with tile.TileContext(nc) as tc:
        with (
            tc.tile_pool(name="const", bufs=1) as cpool,
            tc.tile_pool(name="wload", bufs=2) as wload,
            tc.tile_pool(name="wps", bufs=2, space="PSUM") as wps,
            tc.tile_pool(name="xs", bufs=WARM) as xs,
            tc.tile_pool(name="mps", bufs=6, space="PSUM") as mps,
            tc.tile_pool(name="osb", bufs=8) as osb,
        ):
            # x chunk loads, software-pipelined: issue WARM loads up front
            # (priority follows emission order) so the HBM stream starts
            # immediately and stays ahead of compute.
            chunk_tiles = {}

            def load(ci):
                b, c = divmod(ci, XC)
                t = xs.tile([128, KC, S], mm_dt, tag="xt",
                            name=f"xt_{b}_{c}")
                nc.sync.dma_start(t[:], xT[b][:, c * KC:(c + 1) * KC, :])
                chunk_tiles[ci] = t

            for ci in range(WARM):
                load(ci)

            ident = cpool.tile([R, R], f32, name="ident")
            make_identity(nc, ident[:])

            # Gather the 8 active adapters and transpose to [h, r] layout.
            # wT[:, b*K + k, :] is the [128, 16] stationary operand for
            # batch b, contraction chunk k.
            wT = cpool.tile([128, B * K, R], mm_dt, name="wT")
            pool_rows = pool[:].rearrange("a r h -> (a r) h")
            for b in range(B):
                idx_t = wload.tile([R, 1], i32, tag="idx", name=f"idx_{b}")
                nc.gpsimd.dma_start(idx_t[:], widx[b])
                w_b = wload.tile([R, HS], f32, tag="wb", name=f"wb_{b}")
                nc.gpsimd.indirect_dma_start(
                    out=w_b[:],
                    out_offset=None,
                    in_=pool_rows,
                    in_offset=bass.IndirectOffsetOnAxis(ap=idx_t[:, :1], axis=0),
                )
                for k in range(K):
                    ps = wps.tile([128, R], f32, tag="wt", name=f"wt_{b}_{k}")
                    nc.tensor.transpose(
                        ps[:], w_b[:, k * 128:(k + 1) * 128], ident[:]
                    )
                    nc.vector.tensor_copy(wT[:, b * K + k, :], ps[:])

            # Matmuls accumulate each batch's local contraction into 4 PSUM
            # column strips; each strip is drained (copy + 32 KB store on the
            # scalar/ACT DMA ring) as soon as its accumulation stops.
            psums = None
            for ci in range(NCH):
                b, c = divmod(ci, XC)
                if c == 0:
                    psums = [
                        mps.tile([R, SW], f32, tag="mm", name=f"mm_{b}_{n}")
                        for n in range(NS)
                    ]
                x_t = chunk_tiles.pop(ci)
                last = c == XC - 1
                # last chunk: strip-major so each strip stops (and drains)
                # as early as possible
                order = (
                    [(kc, n) for n in range(NS) for kc in range(KC)]
                    if last else
                    [(kc, n) for kc in range(KC) for n in range(NS)]
                )
                for kc, n in order:
                    k = c * KC + kc
                    nc.tensor.matmul(
                        psums[n][:],
                        lhsT=wT[:, b * K + k, :],
                        rhs=x_t[:, kc, n * SW:(n + 1) * SW],
                        start=(k == 0),
                        stop=(k == K - 1),
                    )
                    if last and kc == KC - 1:
                        o_t = osb.tile([R, SW], f32, tag="ot",
                                       name=f"ot_{b}_{n}")
                        nc.vector.tensor_copy(o_t[:], psums[n][:])
                        nc.scalar.dma_start(
                            out[b][:, n * SW:(n + 1) * SW], o_t[:]
                        )
                if ci + WARM < NCH:
                    load(ci + WARM)
    nc.compile()
    return nc


def _get_nc():
    if MM_DT not in _cache:
        _cache[MM_DT] = _build(MM_DT)
    return _cache[MM_DT]


def _shard_inputs(x, weight, adapter_ids):
    """Host-side sharding: H-slice per core, contraction dim onto partitions."""
    x = np.ascontiguousarray(np.asarray(x, dtype=np.float32))
    weight = np.ascontiguousarray(np.asarray(weight, dtype=np.float32))
    ids = np.asarray(adapter_ids).astype(np.int64)

    # [NCORES, B, 128, K, S]: per-core H-slice of x, laid out so the
    # contraction dim is on partitions (h = k*128 + p) and each partition's
    # data is one contiguous DRAM run per chunk
    xT = np.ascontiguousarray(
        x.reshape(B, S, NCORES, K, 128).transpose(2, 0, 4, 3, 1)
    )
    # [NCORES, POOL, R, HS]: per-core H-slice of the adapter pool
    pool_sh = np.ascontiguousarray(
        weight.reshape(POOL, R, NCORES, HS).transpose(2, 0, 1, 3)
    )
    # row indices into the [(POOL R), HS] flat view: id*R + r
    idx = (ids[:, None] * R + np.arange(R)[None, :]).astype(np.int32)
    idx = np.ascontiguousarray(idx.reshape(B, R, 1))

    return [
        {"xT": xT[d], "pool": pool_sh[d], "widx": idx}
        for d in range(NCORES)
    ]


def _run(x, weight, adapter_ids, trace=False, trace_cores=None):
    from concourse.bass_utils import run_bass_kernel_spmd

    nc = _get_nc()
    in_maps = _shard_inputs(x, weight, adapter_ids)
    res = run_bass_kernel_spmd(
        nc,
        in_maps,
        core_ids=list(range(NCORES)),
        trace=trace,
        trace_cores=trace_cores,
    )
    # Host unshard: sum the 8 partial contractions, restore [B, S, R]
    acc = np.zeros((B, R, S), dtype=np.float64)
    for r in res.results:
        acc += r["out"]
    out = np.ascontiguousarray(acc.transpose(0, 2, 1).astype(np.float32))
    return out, res


def kernel(x, weight, weight_active, adapter_ids):
    # weight_active is all-zeros scratch fully overwritten by the reference's
    # dynamic_update_slice; it does not affect the output.
    out, _ = _run(x, weight, adapter_ids, trace=False)
    return out
